# revision 52
# baseline (speedup 1.0000x reference)
"""Fused pre-LN multi-head attention kernel for Trainium2 (8 NeuronCores).

Problem: B=4, S=2048, D=256, H=8, KD=256.
    out = x_q + MHA(LN(x_q), LN(x_k), LN(x_v))   (keras-style, q scaled 1/sqrt(KD))

Sharding: core c -> batch b = c//2, head group hg = c%2 (4 heads each).
Each core runs a fused flash-style attention over its (batch, 4 heads) and
produces per-head UNNORMALIZED transposed partial outputs plus the softmax
denominators. Host folds LN gamma/beta and all biases into the projection
weights, gathers the 8 cores' partials, divides by the denominators (the
division commutes with the output projection, which contracts kd, not q),
sums heads, and adds residual + constant bias terms.

Device dataflow (all-transposed to keep reductions on friendly axes):
  1. LN stats in natural layout [S,D] (bn_stats/bn_aggr, per-partition ops),
     xhat=(x-mu)*rsqrt(var+eps) cast to bf16.
  2. PE-transpose xhat -> xhatT [D, S].
  3. Projections on PE: qT,kT = W_eff^T @ xhatT (q bias added on DVE),
     v natural = xhat @ Wv_eff.
  4. Per (head, 512-wide q-block): scoresT[k,q] accumulated in PSUM,
     exp on ACT (scores bounded ~|7| here, so no max subtraction) -> bf16
     probs; denominator via ones-matmul into PSUM; PV matmul -> attnT
     (unnormalized, bf16 in SBUF).
  5. Output projection per head (PE) -> fp32 staging -> DMA out per head.
ACT runs exactly two table sets for the whole kernel (sqrt, then exp).
"""

import numpy as np
import ml_dtypes

import concourse.bass as bass
import concourse.bacc as bacc
import concourse.mybir as mybir
import concourse.tile as tile
from concourse.bass_utils import run_bass_kernel_spmd
from concourse.masks import make_identity

B, S, D, H, KD = 4, 2048, 256, 8, 256
HPC = 4              # heads per core
NCORES = 8
EPS = 1e-5
FP = mybir.dt.float32
BF = mybir.dt.bfloat16
AF = mybir.ActivationFunctionType
ALU = mybir.AluOpType

M_HKD = HPC * KD // 128      # 8   qT/kT partition chunks
N_S = S // 512               # 4   512-wide column blocks of S
KB = S // 128                # 16  key-position chunks
DC = D // 128                # 2   contraction / d chunks


def build_program() -> bass.Bass:
    nc = bacc.Bacc(None)

    xq = nc.declare_dram_parameter("xq", [S, D], FP, isOutput=False)
    xk = nc.declare_dram_parameter("xk", [S, D], FP, isOutput=False)
    xv = nc.declare_dram_parameter("xv", [S, D], FP, isOutput=False)
    wq = nc.declare_dram_parameter("wq", [128, DC, HPC * KD], BF, isOutput=False)
    wk = nc.declare_dram_parameter("wk", [128, DC, HPC * KD], BF, isOutput=False)
    wv = nc.declare_dram_parameter("wv", [128, DC, HPC * KD], BF, isOutput=False)
    wo = nc.declare_dram_parameter("wo", [128, HPC, DC, D], BF, isOutput=False)
    bqt = nc.declare_dram_parameter("bqt", [128, M_HKD], FP, isOutput=False)
    out_d = nc.declare_dram_parameter("outT", [HPC, 128, DC, S], FP, isOutput=True)
    den_d = nc.declare_dram_parameter("dens", [HPC * N_S, 512], FP, isOutput=True)

    with tile.TileContext(nc) as tc:
        with (
            tc.tile_pool(name="consts", bufs=1) as consts,
            tc.tile_pool(name="persist", bufs=1) as persist,
            tc.tile_pool(name="probs", bufs=6) as probs_p,
            tc.tile_pool(name="attn_n", bufs=2) as attn_p,
            tc.tile_pool(name="outstage", bufs=4) as out_p,
            tc.tile_pool(name="denstage", bufs=2) as den_p,
        ):
            # ---- constants ----
            ident = consts.tile([128, 128], BF)
            make_identity(nc, ident)
            ones_den = consts.tile([128, 1], BF)
            nc.vector.memset(ones_den, 1.0)
            eps_t = consts.tile([128, 1], FP)
            nc.vector.memset(eps_t, EPS)

            # ---- weights ----
            wq_t = consts.tile([128, DC, HPC * KD], BF)
            wk_t = consts.tile([128, DC, HPC * KD], BF)
            wv_t = consts.tile([128, DC, HPC * KD], BF)
            wo_t = consts.tile([128, HPC, DC, D], BF)
            bqt_t = consts.tile([128, M_HKD], FP)
            def load_weights():
                # Emitted after the q-stream x loads so the input DMAs the
                # LN chain needs are first in the queue. Exactly 8 DMAs to
                # keep the HWDGE lane round-robin aligned with the 8-deep
                # x-load slot recycling.
                nc.sync.dma_start(out=wq_t, in_=wq[:])
                nc.sync.dma_start(out=wk_t, in_=wk[:])
                nc.sync.dma_start(out=wv_t, in_=wv[:])
                nc.sync.dma_start(out=wo_t, in_=wo[:])
                for bi in range(4):
                    nc.sync.dma_start(out=bqt_t[:, bi * 2:(bi + 1) * 2],
                                      in_=bqt[:, bi * 2:(bi + 1) * 2])

            # ---- persistent big tensors ----
            qT = persist.tile([128, M_HKD, S], BF, tag="qT")
            kT = persist.tile([128, M_HKD, S], BF, tag="kT")
            v_t = persist.tile([128, KB, HPC * KD], BF, tag="v")

            # Phase A-C pools live in a nested scope so their SBUF zone is
            # released before the attention phase allocates its buffers.
            with (
                tc.tile_pool(name="xraw", bufs=8) as xraw_p,
                tc.tile_pool(name="ln_small", bufs=8) as ln_small,
                tc.tile_pool(name="xhat", bufs=4) as xhat_p,
                tc.tile_pool(name="xhatT", bufs=1) as xhatT_p,
                tc.tile_pool(name="ps_ac", bufs=6,
                             space=bass.MemorySpace.PSUM) as ps_ac,
            ):
                xhatT = {
                    t: xhatT_p.tile([128, DC, S], BF, name=f"xhatT_{t}",
                                    tag=f"xhatT_{t}")
                    for t in ("q", "k", "v")
                }
                # ---- Phase A+B: LayerNorm + transpose, per stream ----
                for sidx, (tname, xdram) in enumerate(
                        (("k", xk), ("q", xq), ("v", xv))):
                    if sidx == 1:
                        load_weights()
                    xre = xdram.rearrange("(i p) d -> i p d", p=128)
                    for i in range(KB):
                        x_i = xraw_p.tile([128, D], FP, tag="xraw")
                        nc.sync.dma_start(out=x_i, in_=xre[i])
                        stats = ln_small.tile([128, 6], FP, tag="stats")
                        nc.vector.bn_stats(out=stats, in_=x_i)
                        mv = ln_small.tile([128, 2], FP, tag="mv")
                        nc.vector.bn_aggr(out=mv, in_=stats)
                        sd = ln_small.tile([128, 1], FP, tag="sd")
                        # sd = sqrt(var + eps)
                        nc.scalar.activation(
                            out=sd, in_=mv[:, 1:2], func=AF.Sqrt,
                            bias=eps_t[:], scale=1.0,
                        )
                        r = ln_small.tile([128, 1], FP, tag="r")
                        nc.vector.reciprocal(out=r, in_=sd)
                        negmr = ln_small.tile([128, 1], FP, tag="negmr")
                        nc.vector.tensor_scalar(
                            out=negmr, in0=mv[:, 0:1],
                            scalar1=r, scalar2=-1.0,
                            op0=ALU.mult, op1=ALU.mult,
                        )
                        xh = xhat_p.tile([128, D], BF, tag="xh")
                        nc.scalar.activation(
                            out=xh, in_=x_i, func=AF.Identity,
                            bias=negmr[:], scale=r[:],
                        )
                        for j in range(DC):
                            pst = ps_ac.tile([128, 128], BF, name="pst", tag="ac")
                            nc.tensor.transpose(
                                pst, xh[:, j * 128:(j + 1) * 128], ident)
                            dst_sl = xhatT[tname][:, j, i * 128:(i + 1) * 128]
                            if j == 0:
                                nc.vector.tensor_copy(out=dst_sl, in_=pst)
                            else:
                                nc.scalar.copy(out=dst_sl, in_=pst)

                # ---- Phase C: projections ----
                # qT/kT: [HPC*KD, S] = W_eff^T @ xhatT
                for dst, w_t, src, biased in (
                    (kT, wk_t, xhatT["k"], False),
                    (qT, wq_t, xhatT["q"], True),
                ):
                    for m in range(M_HKD):
                        for n in range(N_S):
                            ps = ps_ac.tile([128, 512], FP, tag="ac")
                            for kd in range(DC):
                                nc.tensor.matmul(
                                    ps,
                                    w_t[:, kd, m * 128:(m + 1) * 128],
                                    src[:, kd, n * 512:(n + 1) * 512],
                                    start=(kd == 0), stop=(kd == DC - 1),
                                )
                            dsl = dst[:, m, n * 512:(n + 1) * 512]
                            if biased:
                                nc.scalar.activation(
                                    out=dsl, in_=ps, func=AF.Identity,
                                    bias=bqt_t[:, m:m + 1], scale=1.0)
                            elif n % 2 == 0:
                                nc.scalar.copy(out=dsl, in_=ps)
                            else:
                                nc.vector.tensor_copy(out=dsl, in_=ps)
                # v natural: [S, HPC*KD] = xhat @ Wv_eff
                for i in range(KB):
                    for n in range(HPC * KD // 512):
                        ps = ps_ac.tile([128, 512], FP, tag="ac")
                        for kd in range(DC):
                            nc.tensor.matmul(
                                ps,
                                xhatT["v"][:, kd, i * 128:(i + 1) * 128],
                                wv_t[:, kd, n * 512:(n + 1) * 512],
                                start=(kd == 0), stop=(kd == DC - 1),
                            )
                        if n % 2 == 0:
                            nc.scalar.copy(
                                out=v_t[:, i, n * 512:(n + 1) * 512], in_=ps)
                        else:
                            nc.vector.tensor_copy(
                                out=v_t[:, i, n * 512:(n + 1) * 512], in_=ps)

            # ---- Phase D: attention per (head, q-block) ----
            # attnT and outT stay UNNORMALIZED on device; the softmax
            # denominator commutes with the output projection (it contracts
            # kd; each q column is independent) -> host divides.
            # PSUM: ps_s (4 x 1 bank) + ps_pv (2 banks) + ps_den (1) +
            # ps_po (1) = 8, allocated after the phase A-C pool releases.
            phase_d_pools = (
                tc.tile_pool(name="ps_s", bufs=4, space=bass.MemorySpace.PSUM),
                tc.tile_pool(name="ps_pv", bufs=1, space=bass.MemorySpace.PSUM),
                tc.tile_pool(name="ps_den", bufs=1, space=bass.MemorySpace.PSUM),
                tc.tile_pool(name="ps_po", bufs=1, space=bass.MemorySpace.PSUM),
            )
            ps_s = phase_d_pools[0].__enter__()
            ps_pv = phase_d_pools[1].__enter__()
            ps_den = phase_d_pools[2].__enter__()
            ps_po = phase_d_pools[3].__enter__()
            for h in range(HPC):
                attnT_u = attn_p.tile([128, DC, S], BF, tag="attnT")
                for qb in range(N_S):
                    pv_t = ps_pv.tile([128, DC * 512], FP, name="pv_t", tag="pv")
                    pv = [pv_t[:, m * 512:(m + 1) * 512] for m in range(DC)]
                    den = ps_den.tile([1, 512], FP, tag="den")
                    # Software-pipelined by two kb: scores(kb) and exp(kb)
                    # are emitted two iterations ahead of den/PV(kb) so the
                    # in-order PE queue never stalls on exp latency.
                    prs = [None] * KB
                    for kb in range(KB + 2):
                        if kb < KB:
                            ss = ps_s.tile([128, 512], FP, tag="ss")
                            for kd in range(DC):
                                nc.tensor.matmul(
                                    ss,
                                    kT[:, DC * h + kd, kb * 128:(kb + 1) * 128],
                                    qT[:, DC * h + kd, qb * 512:(qb + 1) * 512],
                                    start=(kd == 0), stop=(kd == DC - 1),
                                )
                            pr = probs_p.tile([128, 512], BF, tag="pr")
                            nc.scalar.activation(out=pr, in_=ss, func=AF.Exp)
                            prs[kb] = pr
                        if kb >= 2:
                            kp = kb - 2
                            nc.tensor.matmul(
                                den, ones_den, prs[kp],
                                start=(kp == 0), stop=(kp == KB - 1),
                            )
                            for m in range(DC):
                                nc.tensor.matmul(
                                    pv[m],
                                    v_t[:, kp, h * KD + m * 128: h * KD + (m + 1) * 128],
                                    prs[kp],
                                    start=(kp == 0), stop=(kp == KB - 1),
                                )
                            prs[kp] = None
                    ds_t = den_p.tile([1, 512], FP, tag="ds")
                    nc.vector.tensor_copy(out=ds_t, in_=den)
                    nc.sync.dma_start(out=den_d[h * N_S + qb], in_=ds_t)
                    for m in range(DC):
                        nc.vector.tensor_copy(
                            out=attnT_u[:, m, qb * 512:(qb + 1) * 512], in_=pv[m],
                        )
                # ---- Phase E: output projection for this head ----
                # Streamed out per 512-column slice so the final head's
                # store overlaps compute instead of sitting in the tail.
                for dc in range(DC):
                    for sb in range(N_S):
                        po = ps_po.tile([128, 512], FP, tag="po")
                        for kd in range(DC):
                            nc.tensor.matmul(
                                po,
                                wo_t[:, h, kd, dc * 128:(dc + 1) * 128],
                                attnT_u[:, kd, sb * 512:(sb + 1) * 512],
                                start=(kd == 0), stop=(kd == DC - 1),
                            )
                        o_t = out_p.tile([128, 512], FP, tag="o")
                        nc.vector.tensor_copy(out=o_t, in_=po)
                        nc.sync.dma_start(
                            out=out_d[h, :, dc, sb * 512:(sb + 1) * 512],
                            in_=o_t)
            for p_ in reversed(phase_d_pools):
                p_.__exit__(None, None, None)

    return nc


_PROG_CACHE = {}


def _get_program() -> bass.Bass:
    if "nc" not in _PROG_CACHE:
        nc = build_program()
        nc.finalize()   # Bacc.compile(): wait splitting, reg alloc, act tables
        _PROG_CACHE["nc"] = nc
    return _PROG_CACHE["nc"]


def _host_prep(input_query, key, value, gq, bq_ln, gk, bk_ln, gv, bv_ln,
               Wq, bq, Wk, bk, Wv, bv, Wo, bo):
    """Fold LN affine + biases into weights; build per-core input maps."""
    bf = ml_dtypes.bfloat16
    scale = np.float32(1.0 / np.sqrt(KD))
    Wq_f = Wq.reshape(D, H * KD).astype(np.float32)
    Wk_f = Wk.reshape(D, H * KD).astype(np.float32)
    Wv_f = Wv.reshape(D, H * KD).astype(np.float32)
    bq_f = bq.reshape(H * KD).astype(np.float32)
    bv_f = bv.reshape(H * KD).astype(np.float32)
    # v-bias flows through softmax (rows sum to 1) -> constant through Wo.
    # k-bias is softmax-invariant (adds a per-q constant to scores) -> dropped.
    bv_eff = bv_ln.astype(np.float32) @ Wv_f + bv_f
    const_full = sum(
        bv_eff[h * KD:(h + 1) * KD] @ Wo[h].astype(np.float32) for h in range(H)
    ) + bo.astype(np.float32)  # [D]

    def chunked(w_eff):  # [D, HPC*KD] -> [128, DC, HPC*KD]
        return np.ascontiguousarray(
            w_eff.reshape(DC, 128, HPC * KD).transpose(1, 0, 2)
        )

    in_maps = []
    for c in range(NCORES):
        b, hg = c // 2, c % 2
        hsl = slice(hg * HPC * KD, (hg + 1) * HPC * KD)
        wq_eff = chunked(((gq[:, None] * Wq_f[:, hsl]) * scale).astype(bf))
        wk_eff = chunked((gk[:, None] * Wk_f[:, hsl]).astype(bf))
        wv_eff = chunked((gv[:, None] * Wv_f[:, hsl]).astype(bf))
        bq_eff = ((bq_ln.astype(np.float32) @ Wq_f[:, hsl] + bq_f[hsl]) * scale)
        bqt_np = np.ascontiguousarray(
            bq_eff.reshape(M_HKD, 128).T.astype(np.float32))          # [128, 8]
        # Wo slice: [128, HPC, DC, D]; [p,h,kd,d] = Wo[hg*4+h][kd*128+p, d]
        wo_np = np.ascontiguousarray(
            Wo[hg * HPC:(hg + 1) * HPC].astype(bf)
            .reshape(HPC, DC, 128, D).transpose(2, 0, 1, 3))
        in_maps.append({
            "xq": np.ascontiguousarray(input_query[b], np.float32),
            "xk": np.ascontiguousarray(key[b], np.float32),
            "xv": np.ascontiguousarray(value[b], np.float32),
            "wq": wq_eff, "wk": wk_eff, "wv": wv_eff,
            "wo": wo_np, "bqt": bqt_np,
        })
    return in_maps, const_full


def kernel(_trace=False, **inputs):
    inputs = {k: np.asarray(v) for k, v in inputs.items()}
    in_maps, const_full = _host_prep(**inputs)
    nc = _get_program()
    res = run_bass_kernel_spmd(nc, in_maps, core_ids=list(range(NCORES)),
                               trace=_trace)
    x_q = inputs["input_query"].astype(np.float32)
    out = np.empty((B, S, D), np.float32)
    for b in range(B):
        acc = np.zeros((S, D), np.float32)
        for hg in range(2):
            r = res.results[2 * b + hg]
            pT = r["outT"]                       # [HPC, 128, DC, S] unnormalized
            dens = r["dens"]                     # [HPC*N_S, 512]
            for h in range(HPC):
                mat = pT[h].transpose(1, 0, 2).reshape(D, S)     # [D, S]
                den = dens[h * N_S:(h + 1) * N_S].reshape(S)     # [S]
                acc += (mat / den[None, :]).T
        out[b] = x_q[b] + const_full[None, :] + acc
    if _trace:
        return out, res
    return out


# revision 57
# speedup vs baseline: 3.4722x; 3.4722x over previous
"""Fused pre-LN multi-head attention kernel for Trainium2 (8 NeuronCores).

Problem: B=4, S=2048, D=256, H=8, KD=256.
    out = x_q + MHA(LN(x_q), LN(x_k), LN(x_v))   (keras-style, q scaled 1/sqrt(KD))

Sharding: core c -> batch b = c//2, head group hg = c%2 (4 heads each).
Each core runs a fused flash-style attention over its (batch, 4 heads) and
produces per-head UNNORMALIZED transposed partial outputs plus the softmax
denominators. Host folds LN gamma/beta and all biases into the projection
weights, gathers the 8 cores' partials, divides by the denominators (the
division commutes with the output projection, which contracts kd, not q),
sums heads, and adds residual + constant bias terms.

Device dataflow (all-transposed to keep reductions on friendly axes):
  1. LN stats in natural layout [S,D] (bn_stats/bn_aggr, per-partition ops),
     xhat=(x-mu)*rsqrt(var+eps) cast to bf16.
  2. PE-transpose xhat -> xhatT [D, S].
  3. Projections on PE: qT,kT = W_eff^T @ xhatT (q bias via ACT Identity),
     v natural = xhat @ Wv_eff.
  4. Per (head, 512-wide q-block): scoresT[k,q] accumulated in PSUM,
     exp on ACT (scores bounded ~|7| here, so no max subtraction) -> bf16
     probs; denominator via ones-matmul into PSUM; PV matmul -> attnT
     (unnormalized, bf16 in SBUF).
  5. Output projection per head (PE) -> fp32 staging -> DMA out per head.
ACT runs exactly two table sets for the whole kernel (sqrt, then exp).
"""

import numpy as np
import ml_dtypes

import concourse.bass as bass
import concourse.bacc as bacc
import concourse.mybir as mybir
import concourse.tile as tile
from concourse.bass_utils import run_bass_kernel_spmd
from concourse.masks import make_identity

B, S, D, H, KD = 4, 2048, 256, 8, 256
HPC = 4              # heads per core
NCORES = 8
EPS = 1e-5
FP = mybir.dt.float32
BF = mybir.dt.bfloat16
AF = mybir.ActivationFunctionType
ALU = mybir.AluOpType

M_HKD = HPC * KD // 128      # 8   qT/kT partition chunks
N_S = S // 512               # 4   512-wide column blocks of S
KB = S // 128                # 16  key-position chunks
DC = D // 128                # 2   contraction / d chunks


def build_program() -> bass.Bass:
    nc = bacc.Bacc(None)

    xq = nc.declare_dram_parameter("xq", [S, D], FP, isOutput=False)
    xk = nc.declare_dram_parameter("xk", [S, D], FP, isOutput=False)
    xv = nc.declare_dram_parameter("xv", [S, D], FP, isOutput=False)
    wq = nc.declare_dram_parameter("wq", [128, DC, HPC * KD], BF, isOutput=False)
    wk = nc.declare_dram_parameter("wk", [128, DC, HPC * KD], BF, isOutput=False)
    wv = nc.declare_dram_parameter("wv", [128, DC, HPC * KD], BF, isOutput=False)
    wo = nc.declare_dram_parameter("wo", [128, HPC, DC, D], BF, isOutput=False)
    bqt = nc.declare_dram_parameter("bqt", [128, M_HKD], FP, isOutput=False)
    out_d = nc.declare_dram_parameter("outT", [HPC, 128, DC, S], FP, isOutput=True)
    den_d = nc.declare_dram_parameter("dens", [HPC * N_S, 512], FP, isOutput=True)

    with tile.TileContext(nc) as tc:
        with (
            tc.tile_pool(name="consts", bufs=1) as consts,
            tc.tile_pool(name="persist", bufs=1) as persist,
            tc.tile_pool(name="probs", bufs=6) as probs_p,
            tc.tile_pool(name="attn_n", bufs=2) as attn_p,
            tc.tile_pool(name="outstage", bufs=1) as out_p,
            tc.tile_pool(name="denstage", bufs=2) as den_p,
        ):
            # ---- constants ----
            ident = consts.tile([128, 128], BF)
            make_identity(nc, ident)
            ones_den = consts.tile([128, 1], BF)
            nc.vector.memset(ones_den, 1.0)
            eps_t = consts.tile([128, 1], FP)
            nc.vector.memset(eps_t, EPS)

            # ---- weights ----
            wq_t = consts.tile([128, DC, HPC * KD], BF)
            wk_t = consts.tile([128, DC, HPC * KD], BF)
            wv_t = consts.tile([128, DC, HPC * KD], BF)
            wo_t = consts.tile([128, HPC, DC, D], BF)
            bqt_t = consts.tile([128, M_HKD], FP)
            def load_weights():
                # Emitted after the q-stream x loads so the input DMAs the
                # LN chain needs are first in the queue. Exactly 8 DMAs to
                # keep the HWDGE lane round-robin aligned with the 8-deep
                # x-load slot recycling.
                nc.sync.dma_start(out=wq_t, in_=wq[:])
                nc.sync.dma_start(out=wk_t, in_=wk[:])
                nc.sync.dma_start(out=wv_t, in_=wv[:])
                nc.sync.dma_start(out=wo_t, in_=wo[:])
                for bi in range(4):
                    nc.sync.dma_start(out=bqt_t[:, bi * 2:(bi + 1) * 2],
                                      in_=bqt[:, bi * 2:(bi + 1) * 2])

            # ---- persistent big tensors ----
            qT = persist.tile([128, M_HKD, S], BF, tag="qT")
            kT = persist.tile([128, M_HKD, S], BF, tag="kT")
            v_t = persist.tile([128, KB, HPC * KD], BF, tag="v")

            # Phase A-C pools live in a nested scope so their SBUF zone is
            # released before the attention phase allocates its buffers.
            with (
                tc.tile_pool(name="xraw", bufs=2) as xraw_p,
                tc.tile_pool(name="ln_small", bufs=8) as ln_small,
                tc.tile_pool(name="xhat", bufs=4) as xhat_p,
                tc.tile_pool(name="xhatT", bufs=1) as xhatT_p,
                tc.tile_pool(name="ps_ac", bufs=6,
                             space=bass.MemorySpace.PSUM) as ps_ac,
            ):
                xhatT = {
                    t: xhatT_p.tile([128, DC, S], BF, name=f"xhatT_{t}",
                                    tag=f"xhatT_{t}")
                    for t in ("q", "k", "v")
                }
                # ---- Phase A+B: LayerNorm + transpose, per stream ----
                for sidx, (tname, xdram) in enumerate(
                        (("k", xk), ("q", xq), ("v", xv))):
                    if sidx == 1:
                        load_weights()
                    xre = xdram.rearrange("(c t p) d -> c p t d", t=8, p=128)
                    xr = None
                    for i in range(KB):
                        c, t = divmod(i, 8)
                        if t == 0:
                            xr = xraw_p.tile([128, 8, D], FP, tag="xraw")
                            nc.sync.dma_start(out=xr, in_=xre[c])
                        x_i = xr[:, t, :]
                        stats = ln_small.tile([128, 6], FP, tag="stats")
                        nc.vector.bn_stats(out=stats, in_=x_i)
                        mv = ln_small.tile([128, 2], FP, tag="mv")
                        nc.vector.bn_aggr(out=mv, in_=stats)
                        sd = ln_small.tile([128, 1], FP, tag="sd")
                        # sd = sqrt(var + eps)
                        nc.scalar.activation(
                            out=sd, in_=mv[:, 1:2], func=AF.Sqrt,
                            bias=eps_t[:], scale=1.0,
                        )
                        r = ln_small.tile([128, 1], FP, tag="r")
                        nc.vector.reciprocal(out=r, in_=sd)
                        negmr = ln_small.tile([128, 1], FP, tag="negmr")
                        nc.vector.tensor_scalar(
                            out=negmr, in0=mv[:, 0:1],
                            scalar1=r, scalar2=-1.0,
                            op0=ALU.mult, op1=ALU.mult,
                        )
                        xh = xhat_p.tile([128, D], BF, tag="xh")
                        nc.scalar.activation(
                            out=xh, in_=x_i, func=AF.Identity,
                            bias=negmr[:], scale=r[:],
                        )
                        for j in range(DC):
                            pst = ps_ac.tile([128, 128], BF, name="pst", tag="ac")
                            nc.tensor.transpose(
                                pst, xh[:, j * 128:(j + 1) * 128], ident)
                            dst_sl = xhatT[tname][:, j, i * 128:(i + 1) * 128]
                            if j == 0:
                                nc.vector.tensor_copy(out=dst_sl, in_=pst)
                            else:
                                nc.scalar.copy(out=dst_sl, in_=pst)

                # ---- Phase C: projections ----
                # qT/kT: [HPC*KD, S] = W_eff^T @ xhatT
                for dst, w_t, src, biased in (
                    (kT, wk_t, xhatT["k"], False),
                    (qT, wq_t, xhatT["q"], True),
                ):
                    for m in range(M_HKD):
                        for n in range(N_S):
                            ps = ps_ac.tile([128, 512], FP, tag="ac")
                            for kd in range(DC):
                                nc.tensor.matmul(
                                    ps,
                                    w_t[:, kd, m * 128:(m + 1) * 128],
                                    src[:, kd, n * 512:(n + 1) * 512],
                                    start=(kd == 0), stop=(kd == DC - 1),
                                )
                            dsl = dst[:, m, n * 512:(n + 1) * 512]
                            if biased:
                                nc.scalar.activation(
                                    out=dsl, in_=ps, func=AF.Identity,
                                    bias=bqt_t[:, m:m + 1], scale=1.0)
                            elif n % 2 == 0:
                                nc.scalar.copy(out=dsl, in_=ps)
                            else:
                                nc.vector.tensor_copy(out=dsl, in_=ps)
                # v natural: [S, HPC*KD] = xhat @ Wv_eff
                for i in range(KB):
                    for n in range(HPC * KD // 512):
                        ps = ps_ac.tile([128, 512], FP, tag="ac")
                        for kd in range(DC):
                            nc.tensor.matmul(
                                ps,
                                xhatT["v"][:, kd, i * 128:(i + 1) * 128],
                                wv_t[:, kd, n * 512:(n + 1) * 512],
                                start=(kd == 0), stop=(kd == DC - 1),
                            )
                        if n % 2 == 0:
                            nc.scalar.copy(
                                out=v_t[:, i, n * 512:(n + 1) * 512], in_=ps)
                        else:
                            nc.vector.tensor_copy(
                                out=v_t[:, i, n * 512:(n + 1) * 512], in_=ps)

            # ---- Phase D: attention per (head, q-block) ----
            # attnT and outT stay UNNORMALIZED on device; the softmax
            # denominator commutes with the output projection (it contracts
            # kd; each q column is independent) -> host divides.
            # PSUM: ps_s (4 x 1 bank) + ps_pv (2 banks) + ps_den (1) +
            # ps_po (1) = 8, allocated after the phase A-C pool releases.
            phase_d_pools = (
                tc.tile_pool(name="ps_s", bufs=4, space=bass.MemorySpace.PSUM),
                tc.tile_pool(name="ps_pv", bufs=1, space=bass.MemorySpace.PSUM),
                tc.tile_pool(name="ps_den", bufs=1, space=bass.MemorySpace.PSUM),
                tc.tile_pool(name="ps_po", bufs=1, space=bass.MemorySpace.PSUM),
            )
            ps_s = phase_d_pools[0].__enter__()
            ps_pv = phase_d_pools[1].__enter__()
            ps_den = phase_d_pools[2].__enter__()
            ps_po = phase_d_pools[3].__enter__()
            for h in range(HPC):
                attnT_u = attn_p.tile([128, DC, S], BF, tag="attnT")
                o_t = out_p.tile([128, DC, S], FP, tag="o")
                for qb in range(N_S):
                    pv_t = ps_pv.tile([128, DC * 512], FP, name="pv_t", tag="pv")
                    pv = [pv_t[:, m * 512:(m + 1) * 512] for m in range(DC)]
                    den = ps_den.tile([1, 512], FP, tag="den")
                    # Software-pipelined by two kb: scores(kb) and exp(kb)
                    # are emitted two iterations ahead of den/PV(kb) so the
                    # in-order PE queue never stalls on exp latency.
                    prs = [None] * KB
                    for kb in range(KB + 2):
                        if kb < KB:
                            ss = ps_s.tile([128, 512], FP, tag="ss")
                            for kd in range(DC):
                                nc.tensor.matmul(
                                    ss,
                                    kT[:, DC * h + kd, kb * 128:(kb + 1) * 128],
                                    qT[:, DC * h + kd, qb * 512:(qb + 1) * 512],
                                    start=(kd == 0), stop=(kd == DC - 1),
                                )
                            pr = probs_p.tile([128, 512], BF, tag="pr")
                            nc.scalar.activation(out=pr, in_=ss, func=AF.Exp)
                            prs[kb] = pr
                        if kb >= 2:
                            kp = kb - 2
                            nc.tensor.matmul(
                                den, ones_den, prs[kp],
                                start=(kp == 0), stop=(kp == KB - 1),
                            )
                            for m in range(DC):
                                nc.tensor.matmul(
                                    pv[m],
                                    v_t[:, kp, h * KD + m * 128: h * KD + (m + 1) * 128],
                                    prs[kp],
                                    start=(kp == 0), stop=(kp == KB - 1),
                                )
                            prs[kp] = None
                    ds_t = den_p.tile([1, 512], FP, tag="ds")
                    nc.vector.tensor_copy(out=ds_t, in_=den)
                    nc.sync.dma_start(out=den_d[h * N_S + qb], in_=ds_t)
                    for m in range(DC):
                        nc.vector.tensor_copy(
                            out=attnT_u[:, m, qb * 512:(qb + 1) * 512], in_=pv[m],
                        )
                    # ---- output projection for this q-block ----
                    for dc in range(DC):
                        po = ps_po.tile([128, 512], FP, tag="po")
                        for kd in range(DC):
                            nc.tensor.matmul(
                                po,
                                wo_t[:, h, kd, dc * 128:(dc + 1) * 128],
                                attnT_u[:, kd, qb * 512:(qb + 1) * 512],
                                start=(kd == 0), stop=(kd == DC - 1),
                            )
                        nc.vector.tensor_copy(
                            out=o_t[:, dc, qb * 512:(qb + 1) * 512], in_=po)
                nc.sync.dma_start(out=out_d[h], in_=o_t)
            for p_ in reversed(phase_d_pools):
                p_.__exit__(None, None, None)

    return nc


_PROG_CACHE = {}


def _get_program() -> bass.Bass:
    if "nc" not in _PROG_CACHE:
        nc = build_program()
        nc.finalize()   # Bacc.compile(): wait splitting, reg alloc, act tables
        _PROG_CACHE["nc"] = nc
    return _PROG_CACHE["nc"]


def _host_prep(input_query, key, value, gq, bq_ln, gk, bk_ln, gv, bv_ln,
               Wq, bq, Wk, bk, Wv, bv, Wo, bo):
    """Fold LN affine + biases into weights; build per-core input maps."""
    bf = ml_dtypes.bfloat16
    scale = np.float32(1.0 / np.sqrt(KD))
    Wq_f = Wq.reshape(D, H * KD).astype(np.float32)
    Wk_f = Wk.reshape(D, H * KD).astype(np.float32)
    Wv_f = Wv.reshape(D, H * KD).astype(np.float32)
    bq_f = bq.reshape(H * KD).astype(np.float32)
    bv_f = bv.reshape(H * KD).astype(np.float32)
    # v-bias flows through softmax (rows sum to 1) -> constant through Wo.
    # k-bias is softmax-invariant (adds a per-q constant to scores) -> dropped.
    bv_eff = bv_ln.astype(np.float32) @ Wv_f + bv_f
    const_full = sum(
        bv_eff[h * KD:(h + 1) * KD] @ Wo[h].astype(np.float32) for h in range(H)
    ) + bo.astype(np.float32)  # [D]

    def chunked(w_eff):  # [D, HPC*KD] -> [128, DC, HPC*KD]
        return np.ascontiguousarray(
            w_eff.reshape(DC, 128, HPC * KD).transpose(1, 0, 2)
        )

    in_maps = []
    for c in range(NCORES):
        b, hg = c // 2, c % 2
        hsl = slice(hg * HPC * KD, (hg + 1) * HPC * KD)
        wq_eff = chunked(((gq[:, None] * Wq_f[:, hsl]) * scale).astype(bf))
        wk_eff = chunked((gk[:, None] * Wk_f[:, hsl]).astype(bf))
        wv_eff = chunked((gv[:, None] * Wv_f[:, hsl]).astype(bf))
        bq_eff = ((bq_ln.astype(np.float32) @ Wq_f[:, hsl] + bq_f[hsl]) * scale)
        bqt_np = np.ascontiguousarray(
            bq_eff.reshape(M_HKD, 128).T.astype(np.float32))          # [128, 8]
        # Wo slice: [128, HPC, DC, D]; [p,h,kd,d] = Wo[hg*4+h][kd*128+p, d]
        wo_np = np.ascontiguousarray(
            Wo[hg * HPC:(hg + 1) * HPC].astype(bf)
            .reshape(HPC, DC, 128, D).transpose(2, 0, 1, 3))
        in_maps.append({
            "xq": np.ascontiguousarray(input_query[b], np.float32),
            "xk": np.ascontiguousarray(key[b], np.float32),
            "xv": np.ascontiguousarray(value[b], np.float32),
            "wq": wq_eff, "wk": wk_eff, "wv": wv_eff,
            "wo": wo_np, "bqt": bqt_np,
        })
    return in_maps, const_full


def kernel(_trace=False, **inputs):
    inputs = {k: np.asarray(v) for k, v in inputs.items()}
    in_maps, const_full = _host_prep(**inputs)
    nc = _get_program()
    res = run_bass_kernel_spmd(nc, in_maps, core_ids=list(range(NCORES)),
                               trace=_trace)
    x_q = inputs["input_query"].astype(np.float32)
    out = np.empty((B, S, D), np.float32)
    for b in range(B):
        acc = np.zeros((S, D), np.float32)
        for hg in range(2):
            r = res.results[2 * b + hg]
            pT = r["outT"]                       # [HPC, 128, DC, S] unnormalized
            dens = r["dens"]                     # [HPC*N_S, 512]
            for h in range(HPC):
                mat = pT[h].transpose(1, 0, 2).reshape(D, S)     # [D, S]
                den = dens[h * N_S:(h + 1) * N_S].reshape(S)     # [S]
                acc += (mat / den[None, :]).T
        out[b] = x_q[b] + const_full[None, :] + acc
    if _trace:
        return out, res
    return out
